# revision 1
# baseline (speedup 1.0000x reference)
"""GCN layer (Linear -> weighted-adjacency SpMM -> BatchNorm(eval) -> exact GELU)
as a Bass/Tile kernel on 8 Trainium2 NeuronCores.

Sharding: destination-node rows are sharded across the 8 cores (12500 rows each);
edges are partitioned by destination row.  W/b/BN params are replicated.  Each core
computes the full `support = x @ W' + b'` redundantly (memory-cheap vs. collectives
on this topology), then gathers source rows of `support` for its own edges with
indirect DMA and segment-sums them on the tensor engine via per-chunk one-hot
selector matmuls (PSUM accumulation).  BN is folded on the host: W' = W * s,
shift = beta - mean * s (s = gamma / sqrt(var + eps)), so the epilogue is one add
+ one Gelu activation.

Host-side prep inside kernel(): transpose x (so matmul lhsT loads need no on-device
transposes), sort/pack edges by destination into fixed 128-edge chunks per 128-row
destination tile, pad chunk counts to a uniform per-tile maximum so one SPMD
program serves all 8 cores.
"""

import sys

sys.path.insert(0, "/opt/trn_rl_repo")

import numpy as np

import concourse.bass as bass
import concourse.tile as tile
from concourse import bacc, mybir
from concourse.bass_utils import run_bass_kernel_spmd

F32 = mybir.dt.float32
I32 = mybir.dt.int32
AF = mybir.ActivationFunctionType
ALU = mybir.AluOpType

N_CORES = 8
TPS = 14          # dest tiles per index-slab load
XCOLS = 512       # node columns per phase-1 supertile


def _build_program(*, in_dim, out_dim, npad, nt, c_max, tps, xcols):
    assert in_dim % 128 == 0 and npad % xcols == 0 and xcols % 128 == 0
    assert nt % tps == 0
    kb = in_dim // 128
    nsup = npad // xcols
    jt = xcols // 128
    nch = nt * c_max
    slab = tps * c_max

    nc = bacc.Bacc("TRN2", target_bir_lowering=False, debug=False,
                   num_devices=N_CORES)

    xT = nc.dram_tensor("xT", [in_dim, npad], F32, kind="ExternalInput").ap()
    Wp = nc.dram_tensor("Wp", [in_dim, out_dim], F32, kind="ExternalInput").ap()
    bp = nc.dram_tensor("bp", [128, out_dim], F32, kind="ExternalInput").ap()
    shiftb = nc.dram_tensor("shiftb", [128, out_dim], F32, kind="ExternalInput").ap()
    iota_in = nc.dram_tensor("iota", [128, 128], F32, kind="ExternalInput").ap()
    idxp = nc.dram_tensor("idxp", [128, nch], I32, kind="ExternalInput").ap()
    rowp = nc.dram_tensor("rowp", [128, nch], F32, kind="ExternalInput").ap()
    valp = nc.dram_tensor("valp", [128, nch], F32, kind="ExternalInput").ap()
    out = nc.dram_tensor("out", [nt * 128, out_dim], F32, kind="ExternalOutput").ap()
    support = nc.dram_tensor("support", [npad, out_dim], F32).ap()

    with tile.TileContext(nc) as tc, tc.tile_pool(name="consts", bufs=1) as consts:
        w_sb = consts.tile([128, kb, out_dim], F32)
        bp_sb = consts.tile([128, out_dim], F32)
        shift_sb = consts.tile([128, out_dim], F32)
        iota_sb = consts.tile([128, 128], F32)
        for k in range(kb):
            nc.sync.dma_start(w_sb[:, k, :], Wp[k * 128:(k + 1) * 128, :])
        nc.sync.dma_start(bp_sb[:], bp[:])
        nc.sync.dma_start(shift_sb[:], shiftb[:])
        nc.sync.dma_start(iota_sb[:], iota_in[:])

        # Phase 1: support = x @ Wp + bp for all (padded) nodes
        with (
            tc.tile_pool(name="xt", bufs=2) as xpool,
            tc.tile_pool(name="p1psum", bufs=4, space="PSUM") as p1psum,
            tc.tile_pool(name="p1out", bufs=4) as p1out,
        ):
            for s in range(nsup):
                xt = xpool.tile([128, kb, xcols], F32)
                for k in range(kb):
                    nc.sync.dma_start(
                        xt[:, k, :],
                        xT[k * 128:(k + 1) * 128, s * xcols:(s + 1) * xcols],
                    )
                for j in range(jt):
                    ps = p1psum.tile([128, out_dim], F32)
                    for k in range(kb):
                        nc.tensor.matmul(
                            ps[:],
                            lhsT=xt[:, k, j * 128:(j + 1) * 128],
                            rhs=w_sb[:, k, :],
                            start=(k == 0),
                            stop=(k == kb - 1),
                        )
                    so = p1out.tile([128, out_dim], F32)
                    nc.vector.tensor_tensor(so[:], ps[:], bp_sb[:], op=ALU.add)
                    n0 = (s * jt + j) * 128
                    nc.sync.dma_start(support[n0:n0 + 128, :], so[:])

        # Phase 2: per dest tile, gather source rows + selector-matmul segment sum
        with (
            tc.tile_pool(name="slabs", bufs=2) as slabs,
            tc.tile_pool(name="gather", bufs=8) as gpool,
            tc.tile_pool(name="sel", bufs=2) as selpool,
            tc.tile_pool(name="p2psum", bufs=4, space="PSUM") as p2psum,
            tc.tile_pool(name="p2out", bufs=4) as opool,
        ):
            for sl in range(nt // tps):
                idx_sb = slabs.tile([128, slab], I32, tag="idx")
                row_sb = slabs.tile([128, slab], F32, tag="row")
                val_sb = slabs.tile([128, slab], F32, tag="val")
                c0 = sl * slab
                nc.sync.dma_start(idx_sb[:], idxp[:, c0:c0 + slab])
                nc.sync.dma_start(row_sb[:], rowp[:, c0:c0 + slab])
                nc.sync.dma_start(val_sb[:], valp[:, c0:c0 + slab])
                for tt in range(tps):
                    t = sl * tps + tt
                    # sel[p, c, d] = (row[p, c] == d) * val[p, c]
                    sel = selpool.tile([128, c_max, 128], F32)
                    row3 = row_sb[:, tt * c_max:(tt + 1) * c_max].unsqueeze(2) \
                        .to_broadcast([128, c_max, 128])
                    val3 = val_sb[:, tt * c_max:(tt + 1) * c_max].unsqueeze(2) \
                        .to_broadcast([128, c_max, 128])
                    iota3 = iota_sb[:].unsqueeze(1).to_broadcast([128, c_max, 128])
                    nc.vector.tensor_tensor(sel[:], row3, iota3, op=ALU.is_equal)
                    nc.vector.tensor_tensor(sel[:], sel[:], val3, op=ALU.mult)
                    ps = p2psum.tile([128, out_dim], F32)
                    for k in range(c_max):
                        gt = gpool.tile([128, out_dim], F32)
                        cg = tt * c_max + k
                        nc.gpsimd.indirect_dma_start(
                            out=gt[:],
                            out_offset=None,
                            in_=support[:],
                            in_offset=bass.IndirectOffsetOnAxis(
                                ap=idx_sb[:, cg:cg + 1], axis=0
                            ),
                        )
                        nc.tensor.matmul(
                            ps[:], lhsT=sel[:, k, :], rhs=gt[:],
                            start=(k == 0), stop=(k == c_max - 1),
                        )
                    ob = opool.tile([128, out_dim], F32, tag="ob")
                    nc.vector.tensor_tensor(ob[:], ps[:], shift_sb[:], op=ALU.add)
                    ob2 = opool.tile([128, out_dim], F32, tag="ob2")
                    nc.scalar.activation(ob2[:], ob[:], AF.Gelu)
                    nc.sync.dma_start(out[t * 128:(t + 1) * 128, :], ob2[:])

    nc.compile()
    return nc


def _preprocess(x, edge_row, edge_col, edge_val, W, b, gamma, beta,
                running_mean, running_var, bn_eps=1e-5):
    n, in_dim = x.shape
    out_dim = W.shape[1]
    npad = ((n + XCOLS - 1) // XCOLS) * XCOLS
    shard = n // N_CORES
    assert shard * N_CORES == n
    nt = (shard + 127) // 128
    nt = ((nt + TPS - 1) // TPS) * TPS

    inv_std = 1.0 / np.sqrt(running_var.astype(np.float64) + bn_eps)
    scale = (inv_std * gamma.astype(np.float64)).astype(np.float32)
    shift = (beta.astype(np.float64)
             - running_mean.astype(np.float64) * inv_std
             * gamma.astype(np.float64)).astype(np.float32)

    xT = np.zeros((in_dim, npad), np.float32)
    xT[:, :n] = np.ascontiguousarray(x.T)
    Wp = (W * scale[None, :]).astype(np.float32)
    bp = np.ascontiguousarray(
        np.broadcast_to((b * scale).astype(np.float32), (128, out_dim)))
    shiftb = np.ascontiguousarray(np.broadcast_to(shift, (128, out_dim)))
    iota = np.ascontiguousarray(
        np.broadcast_to(np.arange(128, dtype=np.float32), (128, 128)))

    per_core = []
    c_max = 1
    for m in range(N_CORES):
        lo, hi = m * shard, (m + 1) * shard
        mask = (edge_row >= lo) & (edge_row < hi)
        er = (edge_row[mask] - lo).astype(np.int64)
        ec = edge_col[mask].astype(np.int32)
        ev = edge_val[mask].astype(np.float32)
        order = np.argsort(er, kind="stable")
        er, ec, ev = er[order], ec[order], ev[order]
        tile_of = er >> 7
        counts = np.bincount(tile_of, minlength=nt)
        per_core.append((er, ec, ev, tile_of, counts))
        c_max = max(c_max, int(((counts + 127) // 128).max()))
    nch = nt * c_max

    in_maps = []
    for m in range(N_CORES):
        er, ec, ev, tile_of, counts = per_core[m]
        starts = np.zeros(nt, np.int64)
        np.cumsum(counts[:-1], out=starts[1:])
        rank = np.arange(len(er)) - starts[tile_of]
        pos = tile_of * (c_max * 128) + rank
        idx_flat = np.zeros(nch * 128, np.int32)
        row_flat = np.zeros(nch * 128, np.float32)
        val_flat = np.zeros(nch * 128, np.float32)
        idx_flat[pos] = ec
        row_flat[pos] = (er & 127).astype(np.float32)
        val_flat[pos] = ev
        in_maps.append({
            "xT": xT, "Wp": Wp, "bp": bp, "shiftb": shiftb, "iota": iota,
            "idxp": np.ascontiguousarray(idx_flat.reshape(nch, 128).T),
            "rowp": np.ascontiguousarray(row_flat.reshape(nch, 128).T),
            "valp": np.ascontiguousarray(val_flat.reshape(nch, 128).T),
        })

    params = dict(in_dim=in_dim, out_dim=out_dim, npad=npad,
                  nt=nt, c_max=c_max, tps=TPS, xcols=XCOLS)
    return in_maps, params, shard


def kernel(x, edge_row, edge_col, edge_val, W, b, gamma, beta,
           running_mean, running_var):
    x = np.asarray(x)
    edge_row = np.asarray(edge_row)
    edge_col = np.asarray(edge_col)
    edge_val = np.asarray(edge_val)
    W = np.asarray(W)
    b = np.asarray(b)
    gamma = np.asarray(gamma)
    beta = np.asarray(beta)
    running_mean = np.asarray(running_mean)
    running_var = np.asarray(running_var)

    in_maps, params, shard = _preprocess(
        x, edge_row, edge_col, edge_val, W, b, gamma, beta,
        running_mean, running_var)
    nc = _build_program(**params)
    res = run_bass_kernel_spmd(nc, in_maps, core_ids=list(range(N_CORES)))
    outs = [res.results[m]["out"][:shard] for m in range(N_CORES)]
    return np.concatenate(outs, axis=0).astype(np.float32)
